# revision 6
# baseline (speedup 1.0000x reference)
"""GCN layer (gnn_message_passing) Trainium2 Bass kernel.

Problem: out[b,n,:] = relu( sum_r (mean_k padded[b, idx[b,r,n,k]]) @ W_r
                            + feat[b,n] @ W_self + bias )
  B=4, N=4096, D=O=128, R=4, K=16.

Strategy: shard (batch x N-half) across 8 cores -> no collectives.
Per core (b, h):
  - DRAM table tbl[4097, 128] bf16 = [zeros; node_features[b]] (host-cast).
  - SWDGE dma_gather (transpose=True) pulls neighbor rows as columns
    [d, j] in bf16. Relation r's gather runs on SWDGE queue r, so the
    descriptor generation for the four relations runs on four different
    Q7 core pairs concurrently (queue q -> cores 2q/2q+1). The idx tile
    holds relation q's stream in partitions [32q, 32q+32).
  - The self rows (consecutive indices n+1) are appended to relation 0's
    stream, so the same gather also delivers featT [d, n] bf16 slices for
    the self matmul; no extra gather or HWDGE transpose needed.
  - DVE tensor_reduce sums k (innermost 16) -> aggT_r [d, n] f32.
  - PE: out_psum[n, o] = sum_r aggT_r_slice.T @ (W_r/K) + featT.T @ W_self
        + ones.T @ bias  (accumulated in PSUM f32).
  - ACT applies ReLU, HWDGE stores [n, o] f32 rows.
"""

import numpy as np
import ml_dtypes

import concourse.bacc as bacc
import concourse.mybir as mybir
from concourse.tile import TileContext
from concourse.bass_utils import run_bass_kernel_spmd

B, N, D = 4, 4096, 128
R, K, O = 4, 16, 128
NCORES = 8
NH = N // 2            # nodes per core
CHUNK = 512            # nodes per chunk
NCH = NH // CHUNK      # chunks per core
RJ = CHUNK * K         # idxs per relation-gather (8192)
RJ0 = RJ + CHUNK       # relation-0 gather also carries the self rows (8704)
SEG = RJ0 // 16        # idx cols per chunk (544)

_cache = {}


def _build():
    nc = bacc.Bacc("TRN2", num_swdge_queues=4)
    tbl = nc.dram_tensor("tbl", [N + 1, D], mybir.dt.bfloat16, kind="ExternalInput")
    idxs = nc.dram_tensor("idxs", [128, NCH * SEG], mybir.dt.int16, kind="ExternalInput")
    w = nc.dram_tensor("w", [128, R + 2, O], mybir.dt.float32, kind="ExternalInput")
    wself = nc.dram_tensor("wself", [128, O], mybir.dt.bfloat16, kind="ExternalInput")
    out = nc.dram_tensor("out", [NH, O], mybir.dt.float32, kind="ExternalOutput")

    with TileContext(nc) as tc:
        with (
            tc.tile_pool(name="const", bufs=1) as cpool,
            tc.tile_pool(name="idx", bufs=2) as ipool,
            tc.tile_pool(name="g", bufs=8) as gpool,
            tc.tile_pool(name="agg", bufs=8) as apool,
            tc.tile_pool(name="osb", bufs=2) as opool,
            tc.tile_pool(name="ps", bufs=8, space="PSUM") as pspool,
        ):
            w_sb = cpool.tile([128, R + 2, O], mybir.dt.float32)
            nc.sync.dma_start(w_sb[:], w[:])
            wself_sb = cpool.tile([128, O], mybir.dt.bfloat16)
            nc.sync.dma_start(wself_sb[:], wself[:])
            ones = cpool.tile([1, 128], mybir.dt.float32)
            nc.vector.memset(ones[:], 1.0)

            prev_g = None
            for ch in range(NCH):
                idx_sb = ipool.tile([128, SEG], mybir.dt.int16)
                nc.sync.dma_start(idx_sb[:], idxs[:, ch * SEG:(ch + 1) * SEG])

                gs = []
                aggs = []
                for r in range(R):
                    nj = RJ0 if r == 0 else RJ
                    g = gpool.tile([128, 1, RJ0], mybir.dt.bfloat16, tag="g")
                    if prev_g is not None:
                        # serialize: gather must wait for previous gather's
                        # data to land (WAW via the corner write).
                        nc.vector.tensor_copy(g[0:1, 0, :16], prev_g[0:1, 0, :16])
                    nc.gpsimd.dma_gather(
                        g[:, :, :nj], tbl[:], idx_sb[:, :nj // 16],
                        nj, nj, D, transpose=True, single_packet=False,
                        queue_num=r,
                    )
                    prev_g = g
                    gs.append(g)
                    aggf = apool.tile([128, CHUNK], mybir.dt.float32, tag="aggf")
                    nc.vector.tensor_reduce(
                        aggf[:],
                        g[:, 0, :RJ].rearrange("p (n k) -> p n k", k=K),
                        mybir.AxisListType.X,
                        mybir.AluOpType.add,
                    )
                    aggs.append(aggf)

                out_sb = opool.tile([128, CHUNK // 128, O], mybir.dt.float32)
                for t in range(CHUNK // 128):
                    ps = pspool.tile([128, O], mybir.dt.float32)
                    sl = slice(t * 128, (t + 1) * 128)
                    ssl = slice(RJ + t * 128, RJ + (t + 1) * 128)
                    for r in range(R):
                        nc.tensor.matmul(
                            ps[:], aggs[r][:, sl], w_sb[:, r, :],
                            start=(r == 0), stop=False,
                        )
                    nc.tensor.matmul(
                        ps[:], gs[0][:, 0, ssl], wself_sb[:],
                        start=False, stop=False,
                    )
                    nc.tensor.matmul(
                        ps[:], ones[:1, :], w_sb[0:1, R + 1, :],
                        start=False, stop=True,
                    )
                    nc.scalar.activation(
                        out_sb[:, t, :], ps[:], mybir.ActivationFunctionType.Relu
                    )
                nc.sync.dma_start(
                    out[ch * CHUNK:(ch + 1) * CHUNK, :].rearrange(
                        "(t p) o -> p t o", p=128
                    ),
                    out_sb[:],
                )

    nc.compile()
    return nc


def _prep_inputs(node_features, neighbor_indices, relation_kernels, self_kernel, bias):
    """Host-side shard/layout prep. Returns per-core input maps."""
    nf = np.asarray(node_features)
    idx = np.asarray(neighbor_indices)
    in_maps = []
    tbls = []
    for b in range(B):
        t = np.zeros((N + 1, D), dtype=ml_dtypes.bfloat16)
        t[1:] = nf[b].astype(ml_dtypes.bfloat16)
        tbls.append(t)

    w = np.zeros((128, R + 2, O), dtype=np.float32)
    for r in range(R):
        w[:, r, :] = np.asarray(relation_kernels)[r] / K
    w[0, R + 1, :] = np.asarray(bias)
    wself = np.asarray(self_kernel).astype(ml_dtypes.bfloat16)

    for c in range(NCORES):
        b, h = divmod(c, 2)
        base = h * NH
        # idx tile: partitions [32q, 32q+32) hold relation q's stream,
        # wrapped 16-wide and duplicated for both Q7 cores of pair q.
        # Relation 0's stream is extended with the 512 self indices.
        cols = np.zeros((128, NCH * SEG), dtype=np.int16)
        for ch in range(NCH):
            lo, hi = base + ch * CHUNK, base + (ch + 1) * CHUNK
            for r in range(R):
                stream = idx[b, r, lo:hi, :].reshape(-1).astype(np.int16)
                if r == 0:
                    selfs = np.arange(lo + 1, hi + 1, dtype=np.int16)
                    stream = np.concatenate([stream, selfs])
                blk = stream.reshape(-1, 16).T
                ncols = blk.shape[1]
                cols[32 * r:32 * r + 16, ch * SEG:ch * SEG + ncols] = blk
                cols[32 * r + 16:32 * r + 32, ch * SEG:ch * SEG + ncols] = blk
        in_maps.append({
            "tbl": tbls[b],
            "idxs": cols,
            "w": w,
            "wself": wself,
        })
    return in_maps


def _run(in_maps, **kw):
    if "nc" not in _cache:
        _cache["nc"] = _build()
    return run_bass_kernel_spmd(_cache["nc"], in_maps, core_ids=list(range(NCORES)), **kw)


def kernel(node_features, neighbor_indices, relation_kernels, self_kernel, bias):
    in_maps = _prep_inputs(node_features, neighbor_indices, relation_kernels,
                           self_kernel, bias)
    res = _run(in_maps)
    out = np.empty((B, N, O), dtype=np.float32)
    for c in range(NCORES):
        b, h = divmod(c, 2)
        out[b, h * NH:(h + 1) * NH, :] = res.results[c]["out"]
    return out


# revision 10
# speedup vs baseline: 2.4753x; 2.4753x over previous
"""GCN layer (gnn_message_passing) Trainium2 Bass kernel.

Problem: out[b,n,:] = relu( sum_r (mean_k padded[b, idx[b,r,n,k]]) @ W_r
                            + feat[b,n] @ W_self + bias )
  B=4, N=4096, D=O=128, R=4, K=16.

Strategy: shard (batch x N-half) across 8 cores -> no collectives.
Per core (b, h):
  - DRAM table tbl[4097, 128] bf16 = [zeros; node_features[b]] (host-cast).
  - SWDGE dma_gather (transpose=False -> no xbar, safe to run queues
    concurrently) pulls neighbor rows into partitions: stream position j
    lands at [j%128, j//128, :]. Relation r's gather runs on SWDGE queue
    r, so descriptor generation for the four relations runs on four Q7
    core pairs concurrently (queue q -> cores 2q/2q+1; trace-verified
    the pairs run ahead through the instruction queue). The idx tile
    holds relation q's stream in partitions [32q, 32q+32).
  - Stream order per chunk: for node block nb (128 nodes), neighbor k of
    node (nb*128+p) sits at column nb*16+k -> k-mean is a strided DVE
    tensor_reduce to agg[p=node, nb, d]. 128 self rows (block t=r) are
    appended to relation r's stream -> g_r[:, 64, :] = self features.
  - PE transpose (identity matmul) flips each [n,d] tile to [d,n] via
    PSUM; ACT copies back to SBUF. Then PE accumulates
    out_psum[n, o] = sum_r aggT_r.T @ (W_r/K) + selfT.T @ W_self + bias.
  - ACT applies ReLU, HWDGE stores [n, o] f32 rows.
"""

import numpy as np
import ml_dtypes

import concourse.bacc as bacc
import concourse.mybir as mybir
from concourse.tile import TileContext
from concourse.bass_utils import run_bass_kernel_spmd

B, N, D = 4, 4096, 128
R, K, O = 4, 16, 128
NCORES = 8
NH = N // 2            # nodes per core
CHUNK = 512            # nodes per chunk
NCH = NH // CHUNK      # chunks per core
NB = CHUNK // 128      # node blocks per chunk (4)
RJ = CHUNK * K         # neighbor idxs per relation-gather (8192)
RJ0 = RJ + 128         # plus 128 self rows (8320)
NCOL = RJ0 // 128      # gather output columns (65)
SEG = RJ0 // 16        # idx cols per chunk (520)

_cache = {}


def _build():
    nc = bacc.Bacc("TRN2", num_swdge_queues=4, dynamic_dma_scratch_size=49152)
    tbl = nc.dram_tensor("tbl", [N + 1, D], mybir.dt.bfloat16, kind="ExternalInput")
    idxs = nc.dram_tensor("idxs", [128, NCH * SEG], mybir.dt.int16, kind="ExternalInput")
    w = nc.dram_tensor("w", [128, R + 2, O], mybir.dt.float32, kind="ExternalInput")
    ident = nc.dram_tensor("ident", [128, 128], mybir.dt.float32, kind="ExternalInput")
    out = nc.dram_tensor("out", [NH, O], mybir.dt.float32, kind="ExternalOutput")

    with TileContext(nc) as tc:
        with (
            tc.tile_pool(name="const", bufs=1) as cpool,
            tc.tile_pool(name="idx", bufs=2) as ipool,
            tc.tile_pool(name="g", bufs=6) as gpool,
            tc.tile_pool(name="agg", bufs=6) as apool,
            tc.tile_pool(name="aggT", bufs=12) as atpool,
            tc.tile_pool(name="osb", bufs=2) as opool,
            tc.tile_pool(name="ps", bufs=4, space="PSUM") as pspool,
            tc.tile_pool(name="acc", bufs=2, space="PSUM") as accpool,
        ):
            w_sb = cpool.tile([128, R + 2, O], mybir.dt.float32)
            nc.sync.dma_start(w_sb[:], w[:])
            id_sb = cpool.tile([128, 128], mybir.dt.float32)
            nc.sync.dma_start(id_sb[:], ident[:])
            ones = cpool.tile([1, 128], mybir.dt.float32)
            nc.vector.memset(ones[:], 1.0)

            for ch in range(NCH):
                idx_sb = ipool.tile([128, SEG], mybir.dt.int16)
                nc.sync.dma_start(idx_sb[:], idxs[:, ch * SEG:(ch + 1) * SEG])

                gs = []
                aggs = []
                selfs = []
                for r in range(R):
                    g = gpool.tile([128, NCOL, D], mybir.dt.bfloat16, tag="g")
                    nc.gpsimd.dma_gather(
                        g[:], tbl[:], idx_sb[:],
                        RJ0, RJ0, D, transpose=False, single_packet=False,
                        queue_num=r,
                    )
                    gs.append(g)
                    # k-mean: [p, nb, k, d] -> sum over k (innermost in AP)
                    aggf = apool.tile([128, NB, D], mybir.dt.float32, tag="aggf")
                    nc.vector.tensor_reduce(
                        aggf[:],
                        g[:, :RJ // 128, :].rearrange("p (nb k) d -> p nb d k", k=K),
                        mybir.AxisListType.X,
                        mybir.AluOpType.add,
                    )
                    aggs.append(aggf)
                    # self rows for node block r: bf16 -> f32
                    sf = apool.tile([128, D], mybir.dt.float32, tag="selff")
                    nc.vector.tensor_copy(sf[:], g[:, RJ // 128, :])
                    selfs.append(sf)

                out_sb = opool.tile([128, NB, O], mybir.dt.float32)
                for t in range(NB):
                    # transpose [n,d] -> [d,n] through PE+PSUM, copy to SBUF
                    tts = []
                    for src in [aggs[r][:, t, :] for r in range(R)] + [selfs[t][:]]:
                        pst = pspool.tile([128, 128], mybir.dt.float32, tag="pst")
                        nc.tensor.transpose(pst[:], src, id_sb[:])
                        tt = atpool.tile([128, 128], mybir.dt.float32, tag="tt")
                        nc.scalar.activation(
                            tt[:], pst[:], mybir.ActivationFunctionType.Copy
                        )
                        tts.append(tt)

                    ps = accpool.tile([128, O], mybir.dt.float32, tag="acc")
                    for r in range(R):
                        nc.tensor.matmul(
                            ps[:], tts[r][:], w_sb[:, r, :],
                            start=(r == 0), stop=False,
                        )
                    nc.tensor.matmul(
                        ps[:], tts[R][:], w_sb[:, R, :],
                        start=False, stop=False,
                    )
                    nc.tensor.matmul(
                        ps[:], ones[:1, :], w_sb[0:1, R + 1, :],
                        start=False, stop=True,
                    )
                    nc.scalar.activation(
                        out_sb[:, t, :], ps[:], mybir.ActivationFunctionType.Relu
                    )
                nc.sync.dma_start(
                    out[ch * CHUNK:(ch + 1) * CHUNK, :].rearrange(
                        "(t p) o -> p t o", p=128
                    ),
                    out_sb[:],
                )

    nc.compile()
    return nc


def _prep_inputs(node_features, neighbor_indices, relation_kernels, self_kernel, bias):
    """Host-side shard/layout prep. Returns per-core input maps."""
    nf = np.asarray(node_features)
    idx = np.asarray(neighbor_indices)
    in_maps = []
    tbls = []
    for b in range(B):
        t = np.zeros((N + 1, D), dtype=ml_dtypes.bfloat16)
        t[1:] = nf[b].astype(ml_dtypes.bfloat16)
        tbls.append(t)

    w = np.zeros((128, R + 2, O), dtype=np.float32)
    for r in range(R):
        w[:, r, :] = np.asarray(relation_kernels)[r] / K
    w[:, R, :] = np.asarray(self_kernel)
    w[0, R + 1, :] = np.asarray(bias)
    ident = np.eye(128, dtype=np.float32)

    for c in range(NCORES):
        b, h = divmod(c, 2)
        base = h * NH
        # idx tile: partitions [32q, 32q+32) hold relation q's stream,
        # wrapped 16-wide and duplicated for both Q7 cores of pair q.
        # Stream order: neighbor k of node (nb*128+p) at position
        # (nb*16+k)*128 + p, then 128 self indices for node block q.
        cols = np.empty((128, NCH * SEG), dtype=np.int16)
        for ch in range(NCH):
            lo = base + ch * CHUNK
            for r in range(R):
                blkidx = idx[b, r, lo:lo + CHUNK, :].astype(np.int16)
                # [nodes=512, k=16] -> [nb=4, 128, 16] -> [nb, k, p]
                stream = blkidx.reshape(NB, 128, K).transpose(0, 2, 1).reshape(-1)
                selfs = np.arange(lo + r * 128 + 1, lo + (r + 1) * 128 + 1,
                                  dtype=np.int16)
                stream = np.concatenate([stream, selfs])
                blk = stream.reshape(-1, 16).T
                cols[32 * r:32 * r + 16, ch * SEG:(ch + 1) * SEG] = blk
                cols[32 * r + 16:32 * r + 32, ch * SEG:(ch + 1) * SEG] = blk
        in_maps.append({
            "tbl": tbls[b],
            "idxs": cols,
            "w": w,
            "ident": ident,
        })
    return in_maps


def _run(in_maps, **kw):
    if "nc" not in _cache:
        _cache["nc"] = _build()
    return run_bass_kernel_spmd(_cache["nc"], in_maps, core_ids=list(range(NCORES)), **kw)


def kernel(node_features, neighbor_indices, relation_kernels, self_kernel, bias):
    in_maps = _prep_inputs(node_features, neighbor_indices, relation_kernels,
                           self_kernel, bias)
    res = _run(in_maps)
    out = np.empty((B, N, O), dtype=np.float32)
    for c in range(NCORES):
        b, h = divmod(c, 2)
        out[b, h * NH:(h + 1) * NH, :] = res.results[c]["out"]
    return out
